# revision 50
# baseline (speedup 1.0000x reference)
"""JPEG-compression kernel for Trainium2 (8 NeuronCores, batch-parallel).

The reference pipeline (rgb2yuv -> 8x8 block DCT -> zigzag mask -> IDCT ->
yuv2rgb) is linear in the image and the zigzag mask is per-channel constant,
so it runs as four chained matmuls with the color conversions folded into
the stage-1/4 matrices and the mask applied as one elementwise multiply.

Zigzag truncation: the kept coefficient set is bounded by k<=6, l<=5 for Y
(25 coeffs) and k<=2, l<=3 for U/V (9 coeffs), so the coefficient domain
only carries N1 = 4*(7+3+3) = 52 (c,a,k) rows and NL = 6 W-frequencies per
8-block (N2 = 96 of 128 columns).  The residual (non-rectangular) part of
the mask is applied in the elementwise multiply.  The truncated chain is
numerically exact vs. the reference math (verified to 1e-15 in fp64).

Data layout per core (4 images):
  partition p = (c, hbl, py) = c*32 + hbl*8 + py   (96 partitions)
  where h = hh*32 + hbl*8 + py, free dim = (hh, w).

Per "pair" (two 32-row groups, hl=0/1; each PSUM tile fits 1-2 banks):
  M1: p1[128, 2*4*52] = X.T @ R1      8 mm, data stationary (transposes;
                                      rgb2yuv + H-DCT; out partitions = w)
  s1: DVE copy p1 -> bf16
  M2: p2[96, 416] = R2.T @ s1         1 mm, R2 stationary, s1 streams
                                      (W-DCT; out partitions = (wbl, l))
  s2: DVE tensor_mul with zigzag mask -> bf16
  M3: p3[52, 512/hl] = s2.T @ R4      8 mm, data stationary (transposes;
                                      W-IDCT; out partitions = (c,a,k))
  s3: ACT/DVE copy p3 -> bf16 (one row-group each)
  M4: p4[96, 512/hl] = R3.T @ s3      2 mm, R3 stationary, s3 streams
                                      (H-IDCT + yuv2rgb; natural layout)
  s4: ACT copy p4 -> s4 f32 (output staging)

Stage 1 of the next pair is emitted one step ahead (software pipelining
against the in-order engine queues).  Input arrives via SWDGE cast-DMAs
(f32->bf16, 6 per image); output leaves via HWDGE f32, flushed per
half-8-row-group, with a finer per-row-group flush at the drain tail.
"""

from contextlib import ExitStack

import ml_dtypes
import numpy as np

NCORES = 8
B, C, H, W = 32, 3, 512, 512
BI = B // NCORES          # images per core
HH = H // 32              # groups of 32 rows
NW = W // 128             # 128-wide w chunks
BLK = 8

KC = (7, 3, 3)            # kept H-frequencies per channel (zigzag bound)
NL = 6                    # kept W-frequencies per 8-block (max over channels)
N1 = 4 * sum(KC)          # 52 coefficient partitions
N2 = 16 * NL              # 96 transformed columns per 128-chunk

_PROGRAM_CACHE = {}


def _build_matrices(D_dct, D_idct, mask):
    """Host-side stage matrices from the kernel inputs."""
    f32 = np.float32
    Dd = np.asarray(D_dct, dtype=f32)
    Di = np.asarray(D_idct, dtype=f32)
    m8 = np.asarray(mask, dtype=f32)[:, :BLK, :BLK]    # (3,8,8) per-channel mask
    Ccv = np.array([[0.299, 0.587, 0.114],
                    [-0.14713, -0.28886, 0.436],
                    [0.615, -0.51499, -0.10001]], dtype=f32)
    Cinv = np.array([[1.0, 0.0, 1.13983],
                     [1.0, -0.39465, -0.5806],
                     [1.0, 2.03211, 0.0]], dtype=f32)

    offs = np.cumsum([0] + [4 * k for k in KC])        # n1 block offsets per c

    R1 = np.zeros((96, N1), dtype=f32)                 # rows (s, a, py)
    for s in range(3):
        for a in range(4):
            for c in range(3):
                for k in range(KC[c]):
                    R1[s * 32 + a * 8:s * 32 + a * 8 + 8,
                       offs[c] + a * KC[c] + k] = Ccv[c, s] * Dd[k, :]

    R2 = np.zeros((128, N2), dtype=f32)                # rows (wbl, px); cols (wbl, l)
    for wbl in range(16):
        for l in range(NL):
            R2[wbl * 8:wbl * 8 + 8, wbl * NL + l] = Dd[l, :]

    # mask rows (wbl, l) -> l; cols (c, a, k) -> (c, k)
    MT = np.zeros((N2, N1), dtype=f32)
    for wbl in range(16):
        for l in range(NL):
            for c in range(3):
                for a in range(4):
                    for k in range(KC[c]):
                        MT[wbl * NL + l, offs[c] + a * KC[c] + k] = m8[c, k, l]

    R3 = np.zeros((N1, 96), dtype=f32)                 # rows (c, a, k); cols (r, b, py)
    for c in range(3):
        for a in range(4):
            for k in range(KC[c]):
                for r in range(3):
                    R3[offs[c] + a * KC[c] + k,
                       r * 32 + a * 8:r * 32 + a * 8 + 8] = Cinv[r, c] * Di[:, k]

    R4 = np.zeros((N2, 128), dtype=f32)                # rows (wbl, l); cols (wbl, px)
    for wbl in range(16):
        for l in range(NL):
            R4[wbl * NL + l, wbl * 8:wbl * 8 + 8] = Di[:, l]

    # mask tile for one pair: [N2, 2 * NW * N1]
    MT2 = np.tile(MT, (1, 2 * NW)).astype(f32)
    # all four stage matrices packed into one [128, 372] constant
    bf16 = ml_dtypes.bfloat16
    CT = np.zeros((128, N1 + N2 + 96 + 128), dtype=np.float32)
    CT[:96, 0:N1] = R1
    CT[:128, N1:N1 + N2] = R2
    CT[:N1, N1 + N2:N1 + N2 + 96] = R3
    CT[:N2, N1 + N2 + 96:] = R4
    return CT.astype(bf16), np.ascontiguousarray(MT2)


def _build_program():
    import concourse.bacc as bacc
    import concourse.tile as tile
    from concourse import mybir

    f32 = mybir.dt.float32
    bf16 = mybir.dt.bfloat16

    nc = bacc.Bacc("TRN2", target_bir_lowering=False, debug=False,
                   enable_asserts=False, num_devices=NCORES)
    x = nc.dram_tensor("x", [BI, C, H, W], f32, kind="ExternalInput").ap()
    ct = nc.dram_tensor("ct", [128, N1 + N2 + 96 + 128], bf16,
                        kind="ExternalInput").ap()
    mt = nc.dram_tensor("mt", [N2, 2 * NW * N1], f32, kind="ExternalInput").ap()
    y = nc.dram_tensor("y", [BI, C, H, W], f32, kind="ExternalOutput").ap()

    with tile.TileContext(nc) as tc:
        with ExitStack() as ctx:
            _emit(ctx, tc, y, x, ct, mt, f32, bf16)
    nc.compile()
    return nc


def _emit(ctx, tc, y, x, ct, mt, f32, bf16):
    nc = tc.nc
    consts = ctx.enter_context(tc.tile_pool(name="consts", bufs=1))
    CT = consts.tile([128, N1 + N2 + 96 + 128], bf16)
    MT2 = consts.tile([N2, 2 * NW * N1], f32)
    nc.sync.dma_start(CT[:], ct)
    nc.sync.dma_start(MT2[:], mt)
    R1 = CT[:96, 0:N1]
    R2 = CT[:, N1:N1 + N2]
    R3 = CT[:N1, N1 + N2:N1 + N2 + 96]
    R4 = CT[:N2, N1 + N2 + 96:]

    xin = ctx.enter_context(tc.tile_pool(name="xin", bufs=3))
    s1p = ctx.enter_context(tc.tile_pool(name="s1", bufs=3))
    s2p = ctx.enter_context(tc.tile_pool(name="s2", bufs=3))
    s3p = ctx.enter_context(tc.tile_pool(name="s3", bufs=3))
    s4p = ctx.enter_context(tc.tile_pool(name="s4", bufs=3))
    # each PSUM tile fits one 2KB bank; bufs=2 keeps two chains in flight
    p1p = ctx.enter_context(tc.tile_pool(name="p1", bufs=2, space="PSUM"))
    p2p = ctx.enter_context(tc.tile_pool(name="p2", bufs=2, space="PSUM"))
    p3p = ctx.enter_context(tc.tile_pool(name="p3", bufs=2, space="PSUM"))
    p4p = ctx.enter_context(tc.tile_pool(name="p4", bufs=2, space="PSUM"))

    xis = {}
    ydsts = {}

    def load_image(i):
        xi = xin.tile([96, HH * W], bf16, name="xi")
        xis[i] = xi
        # DRAM side: [c(3), hp(32) | hh, w] — partition order (c, hbl, py).
        # DMA APs allow at most 3 dims per side, so one DMA per channel.
        src = x[i].rearrange("c (hh hp) w -> c hp hh w", hh=HH, hp=32)
        ydsts[i] = y[i].rearrange("c (q hh hp) w -> c hp q hh w",
                                  q=2, hh=8, hp=32)
        for (ha, hb) in ((0, 8), (8, 16)):
            for c in range(C):
                nc.gpsimd.dma_start(
                    xi[c * 32:(c + 1) * 32,
                       ha * W:hb * W].rearrange(
                        "p (hh w) -> p hh w", hh=hb - ha),
                    src[c, :, ha:hb])              # SWDGE: casts f32 -> bf16

    def stage1(i, q, pair):
        """M1 + s1 for one pair; emitted one step ahead of the rest."""
        xi = xis[i]
        h0 = q * 8 + pair * 2
        p1 = p1p.tile([128, 2 * NW * N1], f32, name="p1t")
        for hl in range(2):
            for wc in range(NW):
                nc.tensor.matmul(
                    p1[:, (hl * NW + wc) * N1:(hl * NW + wc + 1) * N1],
                    xi[:, (h0 + hl) * W + wc * 128:
                       (h0 + hl) * W + (wc + 1) * 128],
                    R1, start=True, stop=True)
        s1 = s1p.tile([128, 2 * NW * N1], bf16, name="s1t")
        nc.vector.tensor_copy(s1[:], p1[:])
        return s1

    steps = [(i, q, pair) for i in range(BI) for q in range(2)
             for pair in range(4)]
    load_image(0)
    s1_next = stage1(*steps[0])
    s4 = None
    for t, (i, q, pair) in enumerate(steps):
        s1 = s1_next
        if q == 0 and pair == 0 and i + 1 < BI:
            load_image(i + 1)      # prefetch a full image ahead
        if pair == 0:
            s4 = s4p.tile([96, 8 * W], f32, name="s4t")
        # software pipeline: emit next pair's M1/s1 ahead of this pair's tail
        if t + 1 < len(steps):
            i2, q2, pair2 = steps[t + 1]
            s1_next = stage1(i2, q2, pair2)
        ydst = ydsts[i]
        # M2': one matmul, R2 stationary, whole pair's s1 streams (N=416)
        p2 = p2p.tile([N2, 2 * NW * N1], f32, name="p2t")
        nc.tensor.matmul(p2[:], R2, s1[:], start=True, stop=True)
        # zigzag mask on the [96, 416] coefficient tile, split per row-group
        # so M3' of the first half starts before the second half is masked
        s2 = s2p.tile([N2, 2 * NW * N1], bf16, name="s2t")
        nc.vector.tensor_mul(s2[:], p2[:], MT2[:])
        # M3': W-IDCT, data stationary (transposing); one bank per row-group
        p3 = [p3p.tile([N1, NW * 128], f32, name="p3t") for hl in range(2)]
        for hl in range(2):
            for wc in range(NW):
                nc.tensor.matmul(
                    p3[hl][:, wc * 128:(wc + 1) * 128],
                    s2[:, (hl * NW + wc) * N1:(hl * NW + wc + 1) * N1],
                    R4, start=True, stop=True)
        # split the two copies across ACT and DVE to balance lane load
        s3 = [s3p.tile([N1, NW * 128], bf16, name="s3t") for hl in range(2)]
        nc.vector.tensor_copy(s3[0][:], p3[0][:])
        nc.scalar.copy(s3[1][:], p3[1][:])
        # M4': H-IDCT, R3 stationary, s3 streams (N=512 per row-group)
        p4 = [p4p.tile([96, NW * 128], f32, name="p4t") for hl in range(2)]
        for hl in range(2):
            nc.tensor.matmul(p4[hl][:], R3, s3[hl][:],
                             start=True, stop=True)
        for hl in range(2):
            nc.scalar.copy(
                s4[:, (pair * 2 + hl) * W:(pair * 2 + hl + 1) * W],
                p4[hl][:])
        last_q = (i == BI - 1 and q == 1)
        if last_q and pair >= 2:
            # drain tail: flush per row-group, all 96 partitions in one
            for hl in range(2):
                hx = pair * 2 + hl
                nc.sync.dma_start(
                    ydst[:, :, q, hx],
                    s4[:, hx * W:(hx + 1) * W])
        elif pair % 2 == 1:
            # flush the finished half of the q-group early
            hf = pair // 2
            for c in range(C):
                nc.sync.dma_start(
                    ydst[c, :, q, hf * 4:(hf + 1) * 4],
                    s4[c * 32:(c + 1) * 32,
                       hf * 4 * W:(hf + 1) * 4 * W].rearrange(
                        "p (hh w) -> p hh w", hh=4))


def kernel(image, D_dct, D_idct, mask):
    from concourse.bass_utils import run_bass_kernel_spmd

    image = np.asarray(image, dtype=np.float32)
    CT, MT2 = _build_matrices(D_dct, D_idct, mask)

    if "prog" not in _PROGRAM_CACHE:
        _PROGRAM_CACHE["prog"] = _build_program()
    nc = _PROGRAM_CACHE["prog"]

    in_maps = []
    for core in range(NCORES):
        in_maps.append({
            "x": np.ascontiguousarray(image[core * BI:(core + 1) * BI]),
            "ct": CT, "mt": MT2,
        })
    res = run_bass_kernel_spmd(nc, in_maps, core_ids=list(range(NCORES)),
                               trace=False)
    _PROGRAM_CACHE["last_result"] = res
    out = np.concatenate([res.results[c]["y"] for c in range(NCORES)], axis=0)
    return out


# revision 64
# speedup vs baseline: 1.0245x; 1.0245x over previous
"""JPEG-compression kernel for Trainium2 (8 NeuronCores, batch-parallel).

The reference pipeline (rgb2yuv -> 8x8 block DCT -> zigzag mask -> IDCT ->
yuv2rgb) is linear in the image and the zigzag mask is per-channel constant,
so it runs as four chained matmuls with the color conversions folded into
the stage-1/4 matrices and the mask applied as one elementwise multiply.

Zigzag truncation: the kept coefficient set is bounded by k<=6, l<=5 for Y
(25 coeffs) and k<=2, l<=3 for U/V (9 coeffs), so the coefficient domain
only carries N1 = 4*(7+3+3) = 52 (c,a,k) rows and NL = 6 W-frequencies per
8-block (N2 = 96 of 128 columns).  The residual (non-rectangular) part of
the mask is applied in the elementwise multiply.  The truncated chain is
numerically exact vs. the reference math (verified to 1e-15 in fp64).

Data layout per core (4 images):
  partition p = (c, hbl, py) = c*32 + hbl*8 + py   (96 partitions)
  where h = hh*32 + hbl*8 + py, free dim = (hh, w).

Per "pair" (two 32-row groups, hl=0/1; each PSUM tile fits 1-2 banks):
  M1: p1[128, 2*4*52] = X.T @ R1      8 mm, data stationary (transposes;
                                      rgb2yuv + H-DCT; out partitions = w)
  s1: DVE copy p1 -> bf16
  M2: p2[96, 416] = R2.T @ s1         1 mm, R2 stationary, s1 streams
                                      (W-DCT; out partitions = (wbl, l))
  s2: DVE tensor_mul with zigzag mask -> bf16
  M3: p3[52, 512/hl] = s2.T @ R4      8 mm, data stationary (transposes;
                                      W-IDCT; out partitions = (c,a,k))
  s3: ACT/DVE copy p3 -> bf16 (one row-group each)
  M4: p4[96, 512/hl] = R3.T @ s3      2 mm, R3 stationary, s3 streams
                                      (H-IDCT + yuv2rgb; natural layout)
  s4: ACT copy p4 -> s4 f32 (output staging)

Stages 1-2 (M1/s1/M2'/mask) are emitted two pairs ahead of stages 3-4
(software pipelining against the in-order engine queues).  Input arrives via SWDGE cast-DMAs
(f32->bf16, 6 per image); output leaves via HWDGE f32, flushed per
half-8-row-group, with a finer per-row-group flush at the drain tail.
"""

from contextlib import ExitStack

import ml_dtypes
import numpy as np

NCORES = 8
B, C, H, W = 32, 3, 512, 512
BI = B // NCORES          # images per core
HH = H // 32              # groups of 32 rows
NW = W // 128             # 128-wide w chunks
BLK = 8

KC = (7, 3, 3)            # kept H-frequencies per channel (zigzag bound)
NL = 6                    # kept W-frequencies per 8-block (max over channels)
N1 = 4 * sum(KC)          # 52 coefficient partitions
N2 = 16 * NL              # 96 transformed columns per 128-chunk

_PROGRAM_CACHE = {}


def _build_matrices(D_dct, D_idct, mask):
    """Host-side stage matrices from the kernel inputs."""
    f32 = np.float32
    Dd = np.asarray(D_dct, dtype=f32)
    Di = np.asarray(D_idct, dtype=f32)
    m8 = np.asarray(mask, dtype=f32)[:, :BLK, :BLK]    # (3,8,8) per-channel mask
    Ccv = np.array([[0.299, 0.587, 0.114],
                    [-0.14713, -0.28886, 0.436],
                    [0.615, -0.51499, -0.10001]], dtype=f32)
    Cinv = np.array([[1.0, 0.0, 1.13983],
                     [1.0, -0.39465, -0.5806],
                     [1.0, 2.03211, 0.0]], dtype=f32)

    offs = np.cumsum([0] + [4 * k for k in KC])        # n1 block offsets per c

    R1 = np.zeros((96, N1), dtype=f32)                 # rows (s, a, py)
    for s in range(3):
        for a in range(4):
            for c in range(3):
                for k in range(KC[c]):
                    R1[s * 32 + a * 8:s * 32 + a * 8 + 8,
                       offs[c] + a * KC[c] + k] = Ccv[c, s] * Dd[k, :]

    R2 = np.zeros((128, N2), dtype=f32)                # rows (wbl, px); cols (wbl, l)
    for wbl in range(16):
        for l in range(NL):
            R2[wbl * 8:wbl * 8 + 8, wbl * NL + l] = Dd[l, :]

    # mask rows (wbl, l) -> l; cols (c, a, k) -> (c, k)
    MT = np.zeros((N2, N1), dtype=f32)
    for wbl in range(16):
        for l in range(NL):
            for c in range(3):
                for a in range(4):
                    for k in range(KC[c]):
                        MT[wbl * NL + l, offs[c] + a * KC[c] + k] = m8[c, k, l]

    R3 = np.zeros((N1, 96), dtype=f32)                 # rows (c, a, k); cols (r, b, py)
    for c in range(3):
        for a in range(4):
            for k in range(KC[c]):
                for r in range(3):
                    R3[offs[c] + a * KC[c] + k,
                       r * 32 + a * 8:r * 32 + a * 8 + 8] = Cinv[r, c] * Di[:, k]

    R4 = np.zeros((N2, 128), dtype=f32)                # rows (wbl, l); cols (wbl, px)
    for wbl in range(16):
        for l in range(NL):
            R4[wbl * NL + l, wbl * 8:wbl * 8 + 8] = Di[:, l]

    # mask tile for one pair: [N2, 2 * NW * N1]
    MT2 = np.tile(MT, (1, 2 * NW)).astype(f32)
    # all four stage matrices packed into one [128, 372] constant
    bf16 = ml_dtypes.bfloat16
    CT = np.zeros((128, N1 + N2 + 96 + 128), dtype=np.float32)
    CT[:96, 0:N1] = R1
    CT[:128, N1:N1 + N2] = R2
    CT[:N1, N1 + N2:N1 + N2 + 96] = R3
    CT[:N2, N1 + N2 + 96:] = R4
    return CT.astype(bf16), np.ascontiguousarray(MT2)


def _build_program():
    import concourse.bacc as bacc
    import concourse.tile as tile
    from concourse import mybir

    f32 = mybir.dt.float32
    bf16 = mybir.dt.bfloat16

    nc = bacc.Bacc("TRN2", target_bir_lowering=False, debug=False,
                   enable_asserts=False, num_devices=NCORES)
    x = nc.dram_tensor("x", [BI, C, H, W], f32, kind="ExternalInput").ap()
    ct = nc.dram_tensor("ct", [128, N1 + N2 + 96 + 128], bf16,
                        kind="ExternalInput").ap()
    mt = nc.dram_tensor("mt", [N2, 2 * NW * N1], f32, kind="ExternalInput").ap()
    y = nc.dram_tensor("y", [BI, C, H, W], f32, kind="ExternalOutput").ap()

    with tile.TileContext(nc) as tc:
        with ExitStack() as ctx:
            _emit(ctx, tc, y, x, ct, mt, f32, bf16)
    nc.compile()
    return nc


def _emit(ctx, tc, y, x, ct, mt, f32, bf16):
    nc = tc.nc
    consts = ctx.enter_context(tc.tile_pool(name="consts", bufs=1))
    CT = consts.tile([128, N1 + N2 + 96 + 128], bf16)
    MT2 = consts.tile([N2, 2 * NW * N1], f32)
    nc.sync.dma_start(CT[:], ct)
    nc.sync.dma_start(MT2[:], mt)
    R1 = CT[:96, 0:N1]
    R2 = CT[:, N1:N1 + N2]
    R3 = CT[:N1, N1 + N2:N1 + N2 + 96]
    R4 = CT[:N2, N1 + N2 + 96:]

    xin = ctx.enter_context(tc.tile_pool(name="xin", bufs=3))
    s1p = ctx.enter_context(tc.tile_pool(name="s1", bufs=2))
    s2p = ctx.enter_context(tc.tile_pool(name="s2", bufs=3))
    s3p = ctx.enter_context(tc.tile_pool(name="s3", bufs=3))
    s4p = ctx.enter_context(tc.tile_pool(name="s4", bufs=3))
    # each PSUM tile fits one 2KB bank; bufs=2 keeps two chains in flight
    p1p = ctx.enter_context(tc.tile_pool(name="p1", bufs=2, space="PSUM"))
    p2p = ctx.enter_context(tc.tile_pool(name="p2", bufs=2, space="PSUM"))
    p3p = ctx.enter_context(tc.tile_pool(name="p3", bufs=2, space="PSUM"))
    p4p = ctx.enter_context(tc.tile_pool(name="p4", bufs=2, space="PSUM"))

    xis = {}
    ydsts = {}

    def load_image(i):
        xi = xin.tile([96, HH * W], bf16, name="xi")
        xis[i] = xi
        # DRAM side: [c(3), hp(32) | hh, w] — partition order (c, hbl, py).
        # DMA APs allow at most 3 dims per side, so one DMA per channel.
        src = x[i].rearrange("c (hh hp) w -> c hp hh w", hh=HH, hp=32)
        ydsts[i] = y[i].rearrange("c (q hh hp) w -> c hp q hh w",
                                  q=2, hh=8, hp=32)
        for (ha, hb) in ((0, 8), (8, 16)):
            for c in range(C):
                nc.gpsimd.dma_start(
                    xi[c * 32:(c + 1) * 32,
                       ha * W:hb * W].rearrange(
                        "p (hh w) -> p hh w", hh=hb - ha),
                    src[c, :, ha:hb])              # SWDGE: casts f32 -> bf16

    def stage12(i, q, pair):
        """M1+s1+M2'+mask for one pair; emitted one step ahead of the rest,
        so DVE's mask never queues behind the previous pair's s3 copy."""
        xi = xis[i]
        h0 = q * 8 + pair * 2
        p1 = p1p.tile([128, 2 * NW * N1], f32, name="p1t")
        for hl in range(2):
            for wc in range(NW):
                nc.tensor.matmul(
                    p1[:, (hl * NW + wc) * N1:(hl * NW + wc + 1) * N1],
                    xi[:, (h0 + hl) * W + wc * 128:
                       (h0 + hl) * W + (wc + 1) * 128],
                    R1, start=True, stop=True)
        s1 = s1p.tile([128, 2 * NW * N1], bf16, name="s1t")
        nc.vector.tensor_copy(s1[:], p1[:])
        # M2': one matmul, R2 stationary, whole pair's s1 streams (N=416)
        p2 = p2p.tile([N2, 2 * NW * N1], f32, name="p2t")
        nc.tensor.matmul(p2[:], R2, s1[:], start=True, stop=True)
        # zigzag mask on the [96, 416] coefficient tile
        s2 = s2p.tile([N2, 2 * NW * N1], bf16, name="s2t")
        nc.vector.tensor_mul(s2[:], p2[:], MT2[:])
        return s2

    steps = [(i, q, pair) for i in range(BI) for q in range(2)
             for pair in range(4)]
    load_image(0)
    s2_queue = [stage12(*steps[0]), stage12(*steps[1])]
    s4 = None
    for t, (i, q, pair) in enumerate(steps):
        s2 = s2_queue.pop(0)
        if q == 0 and pair == 0 and i + 1 < BI:
            load_image(i + 1)      # prefetch a full image ahead
        if pair == 0:
            s4 = s4p.tile([96, 8 * W], f32, name="s4t")
        # software pipeline: emit stages 1-2 two pairs ahead of this tail
        if t + 2 < len(steps):
            s2_queue.append(stage12(*steps[t + 2]))
        ydst = ydsts[i]
        # M3': W-IDCT, data stationary (transposing); one bank per row-group
        p3 = [p3p.tile([N1, NW * 128], f32, name="p3t") for hl in range(2)]
        for hl in range(2):
            for wc in range(NW):
                nc.tensor.matmul(
                    p3[hl][:, wc * 128:(wc + 1) * 128],
                    s2[:, (hl * NW + wc) * N1:(hl * NW + wc + 1) * N1],
                    R4, start=True, stop=True)
        # split the two copies across ACT and DVE to balance lane load
        s3 = [s3p.tile([N1, NW * 128], bf16, name="s3t") for hl in range(2)]
        nc.vector.tensor_copy(s3[0][:], p3[0][:])
        nc.scalar.copy(s3[1][:], p3[1][:])
        # M4': H-IDCT, R3 stationary, s3 streams (N=512 per row-group)
        p4 = [p4p.tile([96, NW * 128], f32, name="p4t") for hl in range(2)]
        for hl in range(2):
            nc.tensor.matmul(p4[hl][:], R3, s3[hl][:],
                             start=True, stop=True)
        for hl in range(2):
            nc.scalar.copy(
                s4[:, (pair * 2 + hl) * W:(pair * 2 + hl + 1) * W],
                p4[hl][:])
        last_q = (i == BI - 1 and q == 1)
        if last_q and pair >= 2:
            # drain tail: flush per row-group, all 96 partitions in one
            for hl in range(2):
                hx = pair * 2 + hl
                nc.sync.dma_start(
                    ydst[:, :, q, hx],
                    s4[:, hx * W:(hx + 1) * W])
        elif pair % 2 == 1:
            # flush the finished half of the q-group early
            hf = pair // 2
            for c in range(C):
                nc.sync.dma_start(
                    ydst[c, :, q, hf * 4:(hf + 1) * 4],
                    s4[c * 32:(c + 1) * 32,
                       hf * 4 * W:(hf + 1) * 4 * W].rearrange(
                        "p (hh w) -> p hh w", hh=4))


def kernel(image, D_dct, D_idct, mask):
    from concourse.bass_utils import run_bass_kernel_spmd

    image = np.asarray(image, dtype=np.float32)
    CT, MT2 = _build_matrices(D_dct, D_idct, mask)

    if "prog" not in _PROGRAM_CACHE:
        _PROGRAM_CACHE["prog"] = _build_program()
    nc = _PROGRAM_CACHE["prog"]

    in_maps = []
    for core in range(NCORES):
        in_maps.append({
            "x": np.ascontiguousarray(image[core * BI:(core + 1) * BI]),
            "ct": CT, "mt": MT2,
        })
    res = run_bass_kernel_spmd(nc, in_maps, core_ids=list(range(NCORES)),
                               trace=False)
    _PROGRAM_CACHE["last_result"] = res
    out = np.concatenate([res.results[c]["y"] for c in range(NCORES)], axis=0)
    return out
